# revision 1
# baseline (speedup 1.0000x reference)
"""Trainium2 Bass kernel for nn_CorrBlock: softmax(fmap1 @ fmap2.T / sqrt(D), axis=-1).

Sharding: fmap1 rows split across 8 cores (1024 rows each), fmap2 replicated.
Each core computes its [1024, 8192] slab of the output independently.

Device kernel (per core):
  - Inputs are pre-transposed on the host to [128, D/128, rows] so the
    contraction dim lands on SBUF partitions with no on-device transpose.
  - PE: matmuls accumulate the D=256 contraction in 2 chunks of 128 into PSUM.
  - ACT: Exp with fused 1/sqrt(D) scale reads PSUM, writes SBUF, and emits
    per-row partial sums via accum_out in the same pass.
  - DVE: reciprocal of the row sum, then per-row scalar multiply (2x mode).
  - DMA out the normalized [128, 8192] block.
"""

import os
import sys

import numpy as np

if "/opt/trn_rl_repo" not in sys.path:
    sys.path.insert(0, "/opt/trn_rl_repo")

import concourse.bacc as bacc
import concourse.bass as bass
import concourse.mybir as mybir
import concourse.tile as tile
from concourse.bass_utils import run_bass_kernel_spmd

N, M, D = 8192, 8192, 256
N_CORES = 8
NB = N // N_CORES  # rows per core
DC = D // 128  # contraction chunks
QC = 2048  # columns handled per PSUM tile (4 banks)

# Matmul input dtype: "float16" halves input DMA bytes and doubles PE rate
# vs "float32r", at ~5e-4 softmax rel err (vs ~2e-4). Both are far inside
# tolerance; float16 wins on the DMA roofline.
MM_DT = os.environ.get("CORR_MM_DT", "float16")

# Populated by kernel() on every run (exec_time_ns only when tracing).
last_run_info: dict = {}


def _chunks(m):
    """Uniform 2048-wide column chunks (4 PSUM banks each). Finer splits
    were tried and regressed: the extra per-ACTIVATE overhead pushed the
    scalar engine past the DMA drain pace and starved the output stream."""
    if m % 2048:
        return [m]
    return [2048] * (m // 2048)


def build_nc(nb=NB, m=M, dc=DC, qc=QC, mm_dt=None, exp_bufs=None):
    """Build the per-core Bass program. Shapes in elements."""
    f32 = mybir.dt.float32
    mm_dtype = getattr(mybir.dt, mm_dt or MM_DT)
    if exp_bufs is None:
        exp_bufs = 4 if mybir.dt.size(mm_dtype) == 2 else 3
    n_blocks = nb // 128
    chunks = _chunks(m)
    n_q = len(chunks)
    coff = [sum(chunks[:i]) for i in range(n_q + 1)]  # column offsets
    scale = 1.0 / (D**0.5)

    nc = bacc.Bacc("TRN2", target_bir_lowering=False, debug=False)

    f1t = nc.dram_tensor("f1t", [128, dc, nb], mm_dtype, kind="ExternalInput")
    f2t = nc.dram_tensor("f2t", [128, dc, m], mm_dtype, kind="ExternalInput")
    out = nc.dram_tensor("out", [nb, m], f32, kind="ExternalOutput")


    with tile.TileContext(nc) as tc:
        with (
            tc.tile_pool(name="weights", bufs=1) as wpool,
            tc.tile_pool(name="exps", bufs=exp_bufs) as epool,
            tc.tile_pool(name="stats", bufs=2) as spool,
            tc.tile_pool(name="psum", bufs=2, space="PSUM") as ppool,
        ):
            f1s = wpool.tile([128, dc, nb], mm_dtype, tag="f1s")
            nc.sync.dma_start(f1s[:], f1t[:])
            f2s = []
            for q in range(n_q):
                f2q = wpool.tile(
                    [128, dc, chunks[q]], mm_dtype, tag=f"f2q_{q}", name=f"f2q_{q}"
                )
                nc.sync.dma_start(f2q[:], f2t[:, :, coff[q] : coff[q + 1]])
                f2s.append(f2q)

            for b in range(n_blocks):
                exps = epool.tile([128, m], f32, tag="exps", name=f"exps_{b}")
                sums = spool.tile([128, n_q], f32, tag="sums", name=f"sums_{b}")
                rsum = spool.tile([128, 1], f32, tag="rsum", name=f"rsum_{b}")
                recip = spool.tile([128, 1], f32, tag="recip", name=f"recip_{b}")
                for q in range(n_q):
                    n_j = chunks[q] // 512
                    ps = ppool.tile([128, n_j, 512], f32, tag="ps", name=f"ps_{b}_{q}")
                    for d in range(dc):
                        lhsT = f1s[:, d, b * 128 : (b + 1) * 128]
                        for j in range(n_j):
                            nc.tensor.matmul(
                                ps[:, j, :],
                                lhsT,
                                f2s[q][:, d, j * 512 : (j + 1) * 512],
                                start=(d == 0),
                                stop=(d == dc - 1),
                            )
                    nc.scalar.activation(
                        exps[:, coff[q] : coff[q + 1]],
                        ps.rearrange("p a b -> p (a b)"),
                        mybir.ActivationFunctionType.Exp,
                        scale=scale,
                        accum_out=sums[:, q : q + 1],
                    )
                nc.vector.reduce_sum(rsum[:], sums[:], axis=mybir.AxisListType.X)
                nc.vector.reciprocal(recip[:], rsum[:])
                for q in range(n_q):
                    sl = slice(coff[q], coff[q + 1])
                    nc.vector.tensor_scalar_mul(exps[:, sl], exps[:, sl], recip[:])
                    nc.sync.dma_start(
                        out[b * 128 : (b + 1) * 128, sl], exps[:, sl]
                    )

    nc.compile()
    return nc


_nc_cache: dict = {}


def _get_nc():
    key = MM_DT
    if key not in _nc_cache:
        _nc_cache[key] = build_nc()
    return _nc_cache[key]


def kernel(fmap1: np.ndarray, fmap2: np.ndarray) -> np.ndarray:
    f1 = np.asarray(fmap1, dtype=np.float32)
    f2 = np.asarray(fmap2, dtype=np.float32)
    np_mm = mybir.dt.np(getattr(mybir.dt, MM_DT))
    # [rows, D] -> [128, D/128, rows]: f1t[dp, dcc, n] = f1[n, dcc*128 + dp]
    f1t = np.ascontiguousarray(
        f1.T.reshape(DC, 128, N).transpose(1, 0, 2).astype(np_mm)
    )
    f2t = np.ascontiguousarray(
        f2.T.reshape(DC, 128, M).transpose(1, 0, 2).astype(np_mm)
    )

    nc = _get_nc()
    in_maps = [
        {"f1t": np.ascontiguousarray(f1t[:, :, i * NB : (i + 1) * NB]), "f2t": f2t}
        for i in range(N_CORES)
    ]
    trace = bool(os.environ.get("BASS_TRACE"))
    res = run_bass_kernel_spmd(nc, in_maps, list(range(N_CORES)), trace=trace)
    last_run_info.clear()
    last_run_info.update(
        exec_time_ns=res.exec_time_ns,
        mean_exec_time_ns=res.mean_exec_time_ns,
        profile_json=res.profile_json,
        trace_path=(res.instructions_and_trace or (None, None))[1],
    )
    return np.concatenate([res.results[i]["out"] for i in range(N_CORES)], axis=0)



# revision 2
# speedup vs baseline: 1.1802x; 1.1802x over previous
"""Trainium2 Bass kernel for nn_CorrBlock: softmax(fmap1 @ fmap2.T / sqrt(D), axis=-1).

Sharding: fmap1 rows split across 8 cores (1024 rows each), fmap2 replicated.
Each core computes its [1024, 8192] slab of the output independently.

Device kernel (per core):
  - Inputs are pre-transposed on the host to [128, D/128, rows] so the
    contraction dim lands on SBUF partitions with no on-device transpose.
  - PE: matmuls accumulate the D=256 contraction in 2 chunks of 128 into PSUM.
  - ACT: Exp with fused 1/sqrt(D) scale reads PSUM, writes SBUF (fp16), and
    emits per-row partial sums via accum_out in the same pass.
  - DVE: reciprocal of the row sum, then per-row scalar multiply (2x fp16 mode).
  - DMA out the normalized [128, 8192] block as fp16; host upcasts to fp32.

The ACT engine is the bottleneck (exp of 8.4M elements/core at ~1 elem/cycle/
partition); everything else (PE, DVE, DMA) is paced to hide under it.
"""

import os
import sys

import numpy as np

if "/opt/trn_rl_repo" not in sys.path:
    sys.path.insert(0, "/opt/trn_rl_repo")

import concourse.bacc as bacc
import concourse.bass as bass
import concourse.mybir as mybir
import concourse.tile as tile
from concourse.bass_utils import run_bass_kernel_spmd

N, M, D = 8192, 8192, 256
N_CORES = 8
NB = N // N_CORES  # rows per core
DC = D // 128  # contraction chunks
QC = 2048  # columns handled per PSUM tile (4 banks)

# Matmul input dtype: "float16" halves input DMA bytes and doubles PE rate
# vs "float32r", at ~5e-4 softmax rel err. Both are far inside tolerance.
MM_DT = os.environ.get("CORR_MM_DT", "float16")
# exp/output dtype: fp16 halves SBUF footprint, DVE time and output DMA bytes.
OUT_DT = os.environ.get("CORR_OUT_DT", "float16")

# Populated by kernel() on every run (exec_time_ns only when tracing).
last_run_info: dict = {}


def _chunks(m):
    """Uniform 2048-wide column chunks (4 PSUM banks each)."""
    if m % 2048:
        return [m]
    return [2048] * (m // 2048)


def build_nc(nb=NB, m=M, dc=DC, qc=QC, mm_dt=None, out_dt=None, exp_bufs=4):
    """Build the per-core Bass program. Shapes in elements."""
    f32 = mybir.dt.float32
    mm_dtype = getattr(mybir.dt, mm_dt or MM_DT)
    out_dtype = getattr(mybir.dt, out_dt or OUT_DT)
    n_blocks = nb // 128
    chunks = _chunks(m)
    n_q = len(chunks)
    coff = [sum(chunks[:i]) for i in range(n_q + 1)]  # column offsets
    scale = 1.0 / (D**0.5)

    nc = bacc.Bacc("TRN2", target_bir_lowering=False, debug=False)

    f1t = nc.dram_tensor("f1t", [128, dc, nb], mm_dtype, kind="ExternalInput")
    f2t = nc.dram_tensor("f2t", [128, dc, m], mm_dtype, kind="ExternalInput")
    out = nc.dram_tensor("out", [nb, m], out_dtype, kind="ExternalOutput")

    with tile.TileContext(nc) as tc:
        with (
            tc.tile_pool(name="weights", bufs=1) as wpool,
            tc.tile_pool(name="exps", bufs=exp_bufs) as epool,
            tc.tile_pool(name="stats", bufs=2) as spool,
            tc.tile_pool(name="psum", bufs=2, space="PSUM") as ppool,
        ):
            # Input DMA priority order: f1 first, then f2 chunk 0 in 512-col
            # pieces (so the first matmul's operands land quickly), then the
            # remaining chunks. The 16 HWDGE rings drain descriptors roughly
            # in issue order, so this sequences the startup traffic.
            f1s = wpool.tile([128, dc, nb], mm_dtype, tag="f1s")
            nc.sync.dma_start(f1s[:], f1t[:])
            f2s = [
                wpool.tile([128, dc, chunks[q]], mm_dtype, tag=f"f2q_{q}", name=f"f2q_{q}")
                for q in range(n_q)
            ]
            for j in range(chunks[0] // 512):
                nc.sync.dma_start(
                    f2s[0][:, :, j * 512 : (j + 1) * 512],
                    f2t[:, :, j * 512 : (j + 1) * 512],
                )
            for q in range(1, n_q):
                nc.sync.dma_start(f2s[q][:], f2t[:, :, coff[q] : coff[q + 1]])

            for b in range(n_blocks):
                exps = epool.tile([128, m], out_dtype, tag="exps", name=f"exps_{b}")
                sums = spool.tile([128, n_q], f32, tag="sums", name=f"sums_{b}")
                rsum = spool.tile([128, 1], f32, tag="rsum", name=f"rsum_{b}")
                recip = spool.tile([128, 1], f32, tag="recip", name=f"recip_{b}")
                for q in range(n_q):
                    n_j = chunks[q] // 512
                    ps = ppool.tile([128, n_j, 512], f32, tag="ps", name=f"ps_{b}_{q}")
                    for d in range(dc):
                        lhsT = f1s[:, d, b * 128 : (b + 1) * 128]
                        for j in range(n_j):
                            nc.tensor.matmul(
                                ps[:, j, :],
                                lhsT,
                                f2s[q][:, d, j * 512 : (j + 1) * 512],
                                start=(d == 0),
                                stop=(d == dc - 1),
                            )
                    nc.scalar.activation(
                        exps[:, coff[q] : coff[q + 1]],
                        ps.rearrange("p a b -> p (a b)"),
                        mybir.ActivationFunctionType.Exp,
                        scale=scale,
                        accum_out=sums[:, q : q + 1],
                    )
                nc.vector.reduce_sum(rsum[:], sums[:], axis=mybir.AxisListType.X)
                nc.vector.reciprocal(recip[:], rsum[:])
                for q in range(n_q):
                    sl = slice(coff[q], coff[q + 1])
                    nc.vector.tensor_scalar_mul(exps[:, sl], exps[:, sl], recip[:])
                    nc.sync.dma_start(
                        out[b * 128 : (b + 1) * 128, sl], exps[:, sl]
                    )

    nc.compile()
    return nc


_nc_cache: dict = {}


def _get_nc():
    key = (MM_DT, OUT_DT)
    if key not in _nc_cache:
        _nc_cache[key] = build_nc()
    return _nc_cache[key]


def kernel(fmap1: np.ndarray, fmap2: np.ndarray) -> np.ndarray:
    f1 = np.asarray(fmap1, dtype=np.float32)
    f2 = np.asarray(fmap2, dtype=np.float32)
    np_mm = mybir.dt.np(getattr(mybir.dt, MM_DT))
    # [rows, D] -> [128, D/128, rows]: f1t[dp, dcc, n] = f1[n, dcc*128 + dp]
    f1t = np.ascontiguousarray(
        f1.T.reshape(DC, 128, N).transpose(1, 0, 2).astype(np_mm)
    )
    f2t = np.ascontiguousarray(
        f2.T.reshape(DC, 128, M).transpose(1, 0, 2).astype(np_mm)
    )

    nc = _get_nc()
    in_maps = [
        {"f1t": np.ascontiguousarray(f1t[:, :, i * NB : (i + 1) * NB]), "f2t": f2t}
        for i in range(N_CORES)
    ]
    trace = bool(os.environ.get("BASS_TRACE"))
    res = run_bass_kernel_spmd(nc, in_maps, list(range(N_CORES)), trace=trace)
    last_run_info.clear()
    last_run_info.update(
        exec_time_ns=res.exec_time_ns,
        mean_exec_time_ns=res.mean_exec_time_ns,
        profile_json=res.profile_json,
        trace_path=(res.instructions_and_trace or (None, None))[1],
    )
    return np.concatenate(
        [res.results[i]["out"] for i in range(N_CORES)], axis=0
    ).astype(np.float32)


# revision 3
# speedup vs baseline: 1.2212x; 1.0348x over previous
"""Trainium2 Bass kernel for nn_CorrBlock: softmax(fmap1 @ fmap2.T / sqrt(D), axis=-1).

Sharding: fmap1 rows split across 8 cores (1024 rows each), fmap2 replicated.
Each core computes its [1024, 8192] slab of the output independently.

Device kernel (per core):
  - Inputs are pre-transposed on the host to [128, D/128, rows] so the
    contraction dim lands on SBUF partitions with no on-device transpose.
  - PE: matmuls accumulate the D=256 contraction in 2 chunks of 128 into PSUM.
  - ACT: Exp with fused 1/sqrt(D) scale reads PSUM, writes SBUF (fp16), and
    emits per-row partial sums via accum_out in the same pass.
  - DVE: reciprocal of the row sum, then per-row scalar multiply (4x fp16 mode).
  - DMA out the normalized [128, 8192] block as fp16; host upcasts to fp32.

The ACT engine is the bottleneck (exp of 8.4M elements/core at ~1 elem/cycle/
partition, ~2.07us per 2048-col chunk incl. accumulator read). The schedule
keeps ACT busy from ~4.5us on:
  - Startup: only f1 + chunk-0 of f2 are DMA'd eagerly, so they get the full
    HBM share and the first matmul starts ~2.5us in. The other f2 chunks are
    released by dummy DVE ops anchored on early compute, staggering their
    transfers behind the startup-critical pieces.
  - Wavefront: chunk-0 EXPs for blocks 0-2 run first (buying ~6us for the
    rest of f2 to arrive), then blocks proceed chunk-major per block, which
    spreads the DVE normalize + output DMA of each block evenly.
"""

import os
import sys

import numpy as np

if "/opt/trn_rl_repo" not in sys.path:
    sys.path.insert(0, "/opt/trn_rl_repo")

import concourse.bacc as bacc
import concourse.bass as bass
import concourse.mybir as mybir
import concourse.tile as tile
from concourse.bass_utils import run_bass_kernel_spmd

N, M, D = 8192, 8192, 256
N_CORES = 8
NB = N // N_CORES  # rows per core
DC = D // 128  # contraction chunks
QC = 2048  # columns per PSUM tile (4 banks); 2 in flight ping-pong

MM_DT = os.environ.get("CORR_MM_DT", "float16")
OUT_DT = os.environ.get("CORR_OUT_DT", "float16")
LEAD = int(os.environ.get("CORR_LEAD", "3"))  # blocks that run chunk-0 first

# Populated by kernel() on every run (exec_time_ns only when tracing).
last_run_info: dict = {}


def build_nc(nb=NB, m=M, dc=DC, qc=QC, mm_dt=None, out_dt=None, exp_bufs=4):
    f32 = mybir.dt.float32
    mm_dtype = getattr(mybir.dt, mm_dt or MM_DT)
    out_dtype = getattr(mybir.dt, out_dt or OUT_DT)
    n_blocks = nb // 128
    n_q = m // qc
    scale = 1.0 / (D**0.5)

    nc = bacc.Bacc("TRN2", target_bir_lowering=False, debug=False)

    f1t = nc.dram_tensor("f1t", [128, dc, nb], mm_dtype, kind="ExternalInput")
    f2t = nc.dram_tensor("f2t", [128, dc, m], mm_dtype, kind="ExternalInput")
    out = nc.dram_tensor("out", [nb, m], out_dtype, kind="ExternalOutput")

    # EXP issue order: chunk-0 for the first LEAD blocks, then per-block
    # chunk-major with the lead blocks finishing their remaining chunks first.
    sched = [(b, 0) for b in range(LEAD)]
    for b in range(LEAD):
        sched += [(b, q) for q in range(1, n_q)]
    for b in range(LEAD, n_blocks):
        sched += [(b, q) for q in range(n_q)]
    assert len(sched) == n_blocks * n_q

    with tile.TileContext(nc) as tc:
        with (
            tc.tile_pool(name="weights", bufs=1) as wpool,
            tc.tile_pool(name="exps", bufs=exp_bufs) as epool,
            tc.tile_pool(name="stats", bufs=4) as spool,
            tc.tile_pool(name="psum", bufs=2, space="PSUM") as ppool,
        ):
            # Eager startup DMAs: f1, then chunk 0 of f2 in 512-col pieces.
            f1s = wpool.tile([128, dc, nb], mm_dtype, tag="f1s")
            nc.sync.dma_start(f1s[:], f1t[:])
            f2s = [
                wpool.tile([128, dc, qc], mm_dtype, tag=f"f2q_{q}", name=f"f2q_{q}")
                for q in range(n_q)
            ]
            for j in range(qc // 512):
                nc.sync.dma_start(
                    f2s[0][:, :, j * 512 : (j + 1) * 512],
                    f2t[:, :, j * 512 : (j + 1) * 512],
                )

            exps = {}
            sums = {}
            started = set()
            deferred_q = set(range(1, n_q))

            def ensure_block(b):
                if b in started:
                    return
                started.add(b)
                exps[b] = epool.tile([128, m], out_dtype, tag="exps", name=f"exps_{b}")
                sums[b] = spool.tile([128, n_q], f32, tag="sums", name=f"sums_{b}")

            def release_f2(q, anchor):
                """Start f2 chunk q's DMA only after `anchor` is produced, so
                startup-critical transfers aren't bandwidth-shared with it.
                The dummy write creates the dependency; the real DMA then
                overwrites the whole tile."""
                nc.vector.tensor_scalar_mul(f2s[q][:, 0, 0:2], anchor, 0.0)
                nc.sync.dma_start(f2s[q][:], f2t[:, :, q * qc : (q + 1) * qc])

            n_done = 0
            for b, q in sched:
                ensure_block(b)
                n_j = qc // 512
                ps = ppool.tile([128, n_j, 512], f32, tag="ps", name=f"ps_{b}_{q}")
                for d in range(dc):
                    lhsT = f1s[:, d, b * 128 : (b + 1) * 128]
                    for j in range(n_j):
                        nc.tensor.matmul(
                            ps[:, j, :],
                            lhsT,
                            f2s[q][:, d, j * 512 : (j + 1) * 512],
                            start=(d == 0),
                            stop=(d == dc - 1),
                        )
                if n_done == 0 and deferred_q:
                    # First anchor: the just-finished first PSUM accumulation.
                    release_f2(min(deferred_q), ps[:, 0, 0:2])
                    deferred_q.discard(min(deferred_q))
                nc.scalar.activation(
                    exps[b][:, q * qc : (q + 1) * qc],
                    ps.rearrange("p a b -> p (a b)"),
                    mybir.ActivationFunctionType.Exp,
                    scale=scale,
                    accum_out=sums[b][:, q : q + 1],
                )
                n_done += 1
                if deferred_q:
                    nq = min(deferred_q)
                    deferred_q.discard(nq)
                    release_f2(nq, exps[b][:, q * qc : q * qc + 2])
                if q == n_q - 1:
                    rsum = spool.tile([128, 1], f32, tag="rsum", name=f"rsum_{b}")
                    recip = spool.tile([128, 1], f32, tag="recip", name=f"recip_{b}")
                    nc.vector.reduce_sum(
                        rsum[:], sums[b][:], axis=mybir.AxisListType.X
                    )
                    nc.vector.reciprocal(recip[:], rsum[:])
                    for qq in range(n_q):
                        sl = slice(qq * qc, (qq + 1) * qc)
                        nc.vector.tensor_scalar_mul(
                            exps[b][:, sl], exps[b][:, sl], recip[:]
                        )
                        nc.sync.dma_start(
                            out[b * 128 : (b + 1) * 128, sl], exps[b][:, sl]
                        )

    nc.compile()
    return nc


_nc_cache: dict = {}


def _get_nc():
    key = (MM_DT, OUT_DT, LEAD)
    if key not in _nc_cache:
        _nc_cache[key] = build_nc()
    return _nc_cache[key]


def kernel(fmap1: np.ndarray, fmap2: np.ndarray) -> np.ndarray:
    f1 = np.asarray(fmap1, dtype=np.float32)
    f2 = np.asarray(fmap2, dtype=np.float32)
    np_mm = mybir.dt.np(getattr(mybir.dt, MM_DT))
    # [rows, D] -> [128, D/128, rows]: f1t[dp, dcc, n] = f1[n, dcc*128 + dp]
    f1t = np.ascontiguousarray(
        f1.T.reshape(DC, 128, N).transpose(1, 0, 2).astype(np_mm)
    )
    f2t = np.ascontiguousarray(
        f2.T.reshape(DC, 128, M).transpose(1, 0, 2).astype(np_mm)
    )

    nc = _get_nc()
    in_maps = [
        {"f1t": np.ascontiguousarray(f1t[:, :, i * NB : (i + 1) * NB]), "f2t": f2t}
        for i in range(N_CORES)
    ]
    trace = bool(os.environ.get("BASS_TRACE"))
    res = run_bass_kernel_spmd(nc, in_maps, list(range(N_CORES)), trace=trace)
    last_run_info.clear()
    last_run_info.update(
        exec_time_ns=res.exec_time_ns,
        mean_exec_time_ns=res.mean_exec_time_ns,
        profile_json=res.profile_json,
        trace_path=(res.instructions_and_trace or (None, None))[1],
    )
    return np.concatenate(
        [res.results[i]["out"] for i in range(N_CORES)], axis=0
    ).astype(np.float32)


# revision 4
# speedup vs baseline: 1.2529x; 1.0259x over previous
"""Trainium2 Bass kernel for nn_CorrBlock: softmax(fmap1 @ fmap2.T / sqrt(D), axis=-1).

Sharding: fmap1 rows split across 8 cores (1024 rows each), fmap2 replicated.
Each core computes its [1024, 8192] slab of the output independently.

Device kernel (per core):
  - Inputs are pre-transposed on the host so the contraction dim lands on SBUF
    partitions with no on-device transpose, and f2 is chunk-major so each
    2048-col chunk is one contiguous 8KB line per partition (128 DMA
    descriptors per chunk — the DMA rings are descriptor-rate limited at
    ~200ns/descriptor, so line size matters more than bytes).
  - PE: matmuls accumulate the D=256 contraction in 2 chunks of 128 into PSUM.
  - ACT: Exp with fused 1/sqrt(D) scale reads PSUM, writes SBUF (fp16), and
    emits per-row partial sums via accum_out in the same pass.
  - DVE: reciprocal of the row sum, then per-row scalar multiply (4x fp16 mode).
  - DMA out the normalized [128, 8192] block as fp16; host upcasts to fp32.

The ACT engine is the bottleneck (exp of 8.4M elements/core at ~1.2G elem/s/
partition; ~1.9us busy per 2048-col chunk, cost = 2048 cycles + SBUF access
init — both confirmed against the instruction cost model and the trace). The
wavefront schedule (chunk-0 EXPs of the first LEAD blocks run first) buys time
for the later f2 chunks to arrive while keeping ACT gapless from ~5us on, and
spreads each block's DVE normalize + output DMA evenly through the run.
"""

import os
import sys

import numpy as np

if "/opt/trn_rl_repo" not in sys.path:
    sys.path.insert(0, "/opt/trn_rl_repo")

import concourse.bacc as bacc
import concourse.bass as bass
import concourse.mybir as mybir
import concourse.tile as tile
from concourse.bass_utils import run_bass_kernel_spmd

N, M, D = 8192, 8192, 256
N_CORES = 8
NB = N // N_CORES  # rows per core
DC = D // 128  # contraction chunks
QC = 2048  # columns per PSUM tile (4 banks); 2 in flight ping-pong
NQ = M // QC

MM_DT = os.environ.get("CORR_MM_DT", "float16")
OUT_DT = os.environ.get("CORR_OUT_DT", "float16")
LEAD = int(os.environ.get("CORR_LEAD", "3"))  # blocks that run chunk-0 first
ACCUM = os.environ.get("CORR_ACCUM", "act")  # act: accum_out; dve: reduce_sum

# Populated by kernel() on every run (exec_time_ns only when tracing).
last_run_info: dict = {}


def build_nc(nb=NB, m=M, dc=DC, qc=QC, mm_dt=None, out_dt=None, exp_bufs=4):
    f32 = mybir.dt.float32
    mm_dtype = getattr(mybir.dt, mm_dt or MM_DT)
    out_dtype = getattr(mybir.dt, out_dt or OUT_DT)
    n_blocks = nb // 128
    n_q = m // qc
    scale = 1.0 / (D**0.5)

    nc = bacc.Bacc("TRN2", target_bir_lowering=False, debug=False)

    f1t = nc.dram_tensor("f1t", [128, dc, nb], mm_dtype, kind="ExternalInput")
    # chunk-major: [partition, chunk, dc, col-in-chunk]
    f2t = nc.dram_tensor("f2t", [128, n_q, dc, qc], mm_dtype, kind="ExternalInput")
    out = nc.dram_tensor("out", [nb, m], out_dtype, kind="ExternalOutput")

    # EXP issue order: chunk-0 for the first LEAD blocks, then per-block
    # chunk-major with the lead blocks finishing their remaining chunks first.
    sched = [(b, 0) for b in range(LEAD)]
    for b in range(LEAD):
        sched += [(b, q) for q in range(1, n_q)]
    for b in range(LEAD, n_blocks):
        sched += [(b, q) for q in range(n_q)]
    assert len(sched) == n_blocks * n_q

    with tile.TileContext(nc) as tc:
        with (
            tc.tile_pool(name="weights", bufs=1) as wpool,
            tc.tile_pool(name="exps", bufs=exp_bufs) as epool,
            tc.tile_pool(name="stats", bufs=4) as spool,
            tc.tile_pool(name="psum", bufs=2, space="PSUM") as ppool,
        ):
            # Input DMAs in priority order; rings drain descriptors FIFO, so
            # f1 + chunk 0 complete first (~256 descriptors, ~4us).
            f1s = wpool.tile([128, dc, nb], mm_dtype, tag="f1s")
            nc.sync.dma_start(f1s[:], f1t[:])
            f2s = []
            for q in range(n_q):
                f2q = wpool.tile([128, dc, qc], mm_dtype, tag=f"f2q_{q}", name=f"f2q_{q}")
                nc.sync.dma_start(f2q[:], f2t[:, q])
                f2s.append(f2q)

            exps = {}
            sums = {}

            for i, (b, q) in enumerate(sched):
                if b not in exps:
                    exps[b] = epool.tile([128, m], out_dtype, tag="exps", name=f"exps_{b}")
                    sums[b] = spool.tile([128, n_q], f32, tag="sums", name=f"sums_{b}")
                n_j = qc // 512
                ps = ppool.tile([128, n_j, 512], f32, tag="ps", name=f"ps_{b}_{q}")
                for d in range(dc):
                    lhsT = f1s[:, d, b * 128 : (b + 1) * 128]
                    for j in range(n_j):
                        nc.tensor.matmul(
                            ps[:, j, :],
                            lhsT,
                            f2s[q][:, d, j * 512 : (j + 1) * 512],
                            start=(d == 0),
                            stop=(d == dc - 1),
                        )
                esl = exps[b][:, q * qc : (q + 1) * qc]
                if ACCUM == "act":
                    nc.scalar.activation(
                        esl,
                        ps.rearrange("p a b -> p (a b)"),
                        mybir.ActivationFunctionType.Exp,
                        scale=scale,
                        accum_out=sums[b][:, q : q + 1],
                    )
                else:
                    nc.scalar.activation(
                        esl,
                        ps.rearrange("p a b -> p (a b)"),
                        mybir.ActivationFunctionType.Exp,
                        scale=scale,
                    )
                    nc.vector.reduce_sum(
                        sums[b][:, q : q + 1], esl, axis=mybir.AxisListType.X
                    )
                if q == n_q - 1:
                    rsum = spool.tile([128, 1], f32, tag="rsum", name=f"rsum_{b}")
                    recip = spool.tile([128, 1], f32, tag="recip", name=f"recip_{b}")
                    nc.vector.reduce_sum(
                        rsum[:], sums[b][:], axis=mybir.AxisListType.X
                    )
                    nc.vector.reciprocal(recip[:], rsum[:])
                    for qq in range(n_q):
                        sl = slice(qq * qc, (qq + 1) * qc)
                        nc.vector.tensor_scalar_mul(
                            exps[b][:, sl], exps[b][:, sl], recip[:]
                        )
                        nc.sync.dma_start(
                            out[b * 128 : (b + 1) * 128, sl], exps[b][:, sl]
                        )

    nc.compile()
    return nc


_nc_cache: dict = {}


def _get_nc():
    key = (MM_DT, OUT_DT, LEAD, ACCUM)
    if key not in _nc_cache:
        _nc_cache[key] = build_nc()
    return _nc_cache[key]


def kernel(fmap1: np.ndarray, fmap2: np.ndarray) -> np.ndarray:
    f1 = np.asarray(fmap1, dtype=np.float32)
    f2 = np.asarray(fmap2, dtype=np.float32)
    np_mm = mybir.dt.np(getattr(mybir.dt, MM_DT))
    # [rows, D] -> [128, D/128, rows]: f1t[dp, dcc, n] = f1[n, dcc*128 + dp]
    f1t = np.ascontiguousarray(
        f1.T.reshape(DC, 128, N).transpose(1, 0, 2).astype(np_mm)
    )
    # [rows, D] -> [128, NQ, D/128, QC]: f2t[dp, q, dcc, c] = f2[q*QC+c, dcc*128+dp]
    f2t = np.ascontiguousarray(
        f2.T.reshape(DC, 128, NQ, QC).transpose(1, 2, 0, 3).astype(np_mm)
    )

    nc = _get_nc()
    in_maps = [
        {"f1t": np.ascontiguousarray(f1t[:, :, i * NB : (i + 1) * NB]), "f2t": f2t}
        for i in range(N_CORES)
    ]
    trace = bool(os.environ.get("BASS_TRACE"))
    res = run_bass_kernel_spmd(nc, in_maps, list(range(N_CORES)), trace=trace)
    last_run_info.clear()
    last_run_info.update(
        exec_time_ns=res.exec_time_ns,
        mean_exec_time_ns=res.mean_exec_time_ns,
        profile_json=res.profile_json,
        trace_path=(res.instructions_and_trace or (None, None))[1],
    )
    return np.concatenate(
        [res.results[i]["out"] for i in range(N_CORES)], axis=0
    ).astype(np.float32)
